# revision 28
# baseline (speedup 1.0000x reference)
"""BERT self-attention Bass/Tile kernel for 8 Trainium2 NeuronCores.

Problem: hidden [2, 2048, 768], 12 heads x 64 dim, additive mask [2,1,1,2048].
Sharding: batch x head-group. Core c handles batch b = c // 4 and global heads
3*(c%4) .. 3*(c%4)+2 (columns 192*(c%4) .. +192 of Wq/Wk/Wv).  Each core
computes its 3 heads' full attention locally; outputs are concatenated on the
host (no cross-device communication).

Engine model (measured on HW): the PE streams matmul output columns at
1 col/cycle (2.4 GHz); ScalarE exp runs at ~1.09 ns/elem and its 3 heads x
2048^2 scores = 107us is the binding engine.  The schedule therefore feeds
ScalarE from ~8.5us and keeps it saturated; all PE work (~91us of columns)
hides underneath.  Staying under ~130us also avoids the DVFS util-limit
(0.5x PE clock) that bit the previous 177us version.

Per-core pipeline (one TileContext):
  X [2048,768] --cast fp16, PE transpose--> X_T [768, 2048]       (12.3k cols)
  Q_T/K_T pairs = W.T @ X_T  (heads 0,1 packed M=128)             (24.6k cols)
  [q_h2|k_h2]  = one M=128 matmul vs combined weight tile         (12.3k cols)
  V directly in [k, d] layout: X_T chunk stationary, Wv moving,
     bias via a K=1 ones-row matmul; no V transpose, no M=64 solo (21.5k cols)
  scores unit (h, J, g): 2 x N=512 matmuls (kc=2g,2g+1) K=64
     into one [128,1024] PSUM pair of banks                       (98.3k cols)
  probs = exp(scores/8): ONE ScalarE activation per unit, fp16
  ctx chains (h, J, s): 16 x N=65 matmuls accumulate probs.T @ V_aug;
     col 64 = softmax denominator (e-column of V_aug)             (49.9k cols)
  out[q, d] = ctx[:, :64] * (1 / ctx[:, 64])   -> DMA to DRAM

The additive mask folds into V: exp(s + m_k) = exp(s) * exp(m_k); both the
numerator and denominator columns of V_aug are pre-scaled by exp(m_k) (a
per-partition scalar in the [k, d] layout).  All-zero mask skips the scale
and memsets the denominator column to 1.
"""

import numpy as np

import concourse.bass as bass
import concourse.tile as tile
from concourse import bacc, mybir
from concourse.bass_utils import run_bass_kernel_spmd
from concourse.masks import make_identity

F32 = mybir.dt.float32
F16 = mybir.dt.float16
EXP = mybir.ActivationFunctionType.Exp

S = 2048           # sequence length
DM = 768           # model dim
DH = 64            # head dim
NHL = 3            # local heads per core
FC = DM // 128     # 6 f-chunks (contraction for projections)
KC = S // 128      # 16 k-chunks
QB = 512           # q block width (J indexes blocks of 512)
NQB = S // QB      # 4 q blocks
NG = KC // 2       # 8 k-chunk groups per (h, J); one exp per (h, J, g)


def _build_kernel(zero_mask: bool) -> bass.Bass:
    nc = bacc.Bacc()

    x_d = nc.declare_dram_parameter("x", [S, DM], F32, isOutput=False)
    wq_d = nc.declare_dram_parameter("wq", [DM, 192], F32, isOutput=False)
    wk_d = nc.declare_dram_parameter("wk", [DM, 192], F32, isOutput=False)
    wv_d = nc.declare_dram_parameter("wv", [DM, 192], F32, isOutput=False)
    bq_d = nc.declare_dram_parameter("bq", [192], F32, isOutput=False)
    bk_d = nc.declare_dram_parameter("bk", [192], F32, isOutput=False)
    bv_d = nc.declare_dram_parameter("bv", [192], F32, isOutput=False)
    m_d = nc.declare_dram_parameter("mask", [S], F32, isOutput=False)
    out_d = nc.declare_dram_parameter("out", [S, 192], F32, isOutput=True)

    with tile.TileContext(nc) as tc:
        _attention(tc, x_d, (wq_d, wk_d, wv_d), (bq_d, bk_d, bv_d), m_d, out_d,
                   zero_mask)
    nc.compile()
    return nc


def _attention(tc, x_d, w_ds, b_ds, m_d, out_d, zero_mask):
    nc = tc.nc

    const = tc.alloc_tile_pool(name="const", bufs=1)
    xpool = tc.alloc_tile_pool(name="xpool", bufs=5)
    persist = tc.alloc_tile_pool(name="persist", bufs=1)
    probs_pool = tc.alloc_tile_pool(name="probs", bufs=40)
    small = tc.alloc_tile_pool(name="small", bufs=4)
    outp = tc.alloc_tile_pool(name="outp", bufs=1)
    ps = tc.alloc_tile_pool(name="ps", bufs=2, space="PSUM")

    # --- constants: identity first (cheap, gates transposes) ----------------
    ident16 = const.tile([128, 128], F16)
    make_identity(nc, ident16)
    ones_row = const.tile([1, 128], F16)
    nc.gpsimd.memset(ones_row, 1.0)
    ones512 = const.tile([1, 512], F16)
    nc.gpsimd.memset(ones512, 1.0)

    # Block-0 X loads go first on the sync queue + DVE so the transposes (the
    # longest warmup chain) start as early as possible.
    x16s_0 = _load_x_block_fns[0]() if False else None  # placeholder

    # one DMA per weight matrix: [768,192] -> [128, 6*192]; K first (the
    # first score units need K earliest), then Q, then V.  All biases load as
    # contiguous [1,192] rows (a per-partition gather would cost ~128
    # descriptors and poison early DMA bandwidth) and are applied as K=1
    # ones-matmuls into PSUM.
    def _load_weights():
        w16l = []   # t -> [128, 1152] fp16, f-chunk f at cols 192f..192f+192
        for t in (1, 0, 2):
            w32 = small.tile([128, FC * 192], F32, name=f"w32_{t}", tag="w32",
                             bufs=2)
            nc.gpsimd.dma_start(
                out=w32[:, :].rearrange("p (f j) -> p f j", j=192),
                in_=w_ds[t][:, :].rearrange("(f p) j -> p f j", p=128))
            wt = const.tile([128, FC * 192], F16, name=f"w16_{t}")
            nc.vector.tensor_copy(out=wt, in_=w32)
            w16l.append(wt)
        return [w16l[1], w16l[0], w16l[2]]  # back to q, k, v order

    # --- persistent projection outputs --------------------------------------
    # QT2/KT2: [128, 2048] fp16, rows 0:64 = head0, 64:128 = head1
    # QTs/KTs: head2 at rows 0:64 only (single-stream score units)
    XT = [persist.tile([128, S], F16, name=f"XT_{f}") for f in range(FC)]
    QT2 = persist.tile([128, S], F16)
    KT2 = persist.tile([128, S], F16)
    QTs = persist.tile([64, S], F16)
    KTs = persist.tile([128, S], F16)
    # V[kc] layout: [V_h0(64) | e | V_h1(64) | e | V_h2(64) | e], e = exp(m_k)
    V = [persist.tile([128, 195], F16, name=f"V_{kc}") for kc in range(KC)]
    # Vs[kc] = V[kc] with partition halves swapped: the quadrant-paired ctx
    # matmuls need each kpos half readable from both partition ranges.
    Vs = [persist.tile([128, 195], F16, name=f"Vs_{kc}") for kc in range(KC)]

    out_tiles = [outp.tile([128, 192], F32, name=f"o_{u}") for u in range(16)]
    out_written = [0] * 16

    def load_x_block(m):
        """DMA 4 q-tiles of X, cast fp16 on DVE."""
        x16s = []
        for j in range(4):
            qt = 4 * m + j
            xt32 = xpool.tile([128, DM], F32, name=f"x_{qt}", tag="x", bufs=3)
            nc.sync.dma_start(out=xt32, in_=x_d[128 * qt:128 * (qt + 1), :])
            x16 = xpool.tile([128, DM], F16, name=f"x16_{qt}", tag="x16")
            nc.vector.tensor_copy(out=x16, in_=xt32)
            x16s.append(x16)
        return x16s

    def transpose_block(m, x16s):
        """PE-transpose the 4 fp16 q-tiles into XT[f][:, 512m:512m+512]."""
        for f in range(FC):
            tp = ps.tile([128, 512], F16, name=f"xt_ps_{m}_{f}", tag="mid",
                         bufs=2)
            for j in range(4):
                nc.tensor.transpose(
                    tp[:, 128 * j:128 * (j + 1)],
                    x16s[j][:, 128 * f:128 * (f + 1)],
                    ident16,
                )
            nc.vector.tensor_copy(out=XT[f][:, 512 * m:512 * (m + 1)], in_=tp)

    def proj_pair(t, dst_pair, m):
        cols = slice(512 * m, 512 * (m + 1))
        pp = ps.tile([128, 512], F32, name=f"proj_{t}_{m}", tag="mid", bufs=2)
        for f in range(FC):
            nc.tensor.matmul(pp, w16[t][:, 192 * f:192 * f + 128],
                             XT[f][:, cols],
                             start=(f == 0), stop=(f == FC - 1))
        nc.vector.tensor_scalar_add(out=dst_pair[:, cols], in0=pp,
                                    scalar1=bias_pair[t])

    def proj_solo(m):
        """q_h2 (psum rows 0:64) and k_h2 (rows 64:128) in one M=128 chain.
        k_h2 is then partition-shifted to KTs rows 0:64 by DMA so the head-2
        score matmuls read both operands at base partition 0."""
        cols = slice(512 * m, 512 * (m + 1))
        sp = ps.tile([128, 512], F32, name=f"proj_s_{m}", tag="mid", bufs=2)
        for f in range(FC):
            nc.tensor.matmul(sp, wsolo[f], XT[f][:, cols],
                             start=(f == 0), stop=(f == FC - 1))
        nc.vector.tensor_scalar_add(out=QTs[0:64, cols], in0=sp[0:64],
                                    scalar1=bias_solo[0:64])
        nc.vector.tensor_scalar_add(out=KTs[64:128, cols], in0=sp[64:128],
                                    scalar1=bias_solo[64:128])
        nc.sync.dma_start(out=KTs[0:64, cols], in_=KTs[64:128, cols])

    def build_v(kc):
        """V[kc] [k, d] directly: X_T chunks stationary, Wv moving; bias via
        the ones-row K=1 matmul; then fp16 copy (+ mask scale) into V_aug."""
        u = kc  # seq-tile index
        vp = ps.tile([128, 192], F32, name=f"v_ps_{kc}", tag="mid", bufs=2)
        nc.tensor.matmul(vp, ones_row, bv16, start=True, stop=False)
        for f in range(FC):
            nc.tensor.matmul(vp, XT[f][:, 128 * u:128 * (u + 1)],
                             w16[2][:, 192 * f:192 * f + 192],
                             start=False, stop=(f == FC - 1))
        ecol = bass.AP(tensor=V[kc].tensor, offset=V[kc].offset + 64,
                       ap=[V[kc].ap[0], [65, 3]])
        vdst = bass.AP(tensor=V[kc].tensor, offset=V[kc].offset,
                       ap=[V[kc].ap[0], [65, 3], [1, 64]])
        vsrc = vp[:, 0:192].rearrange("p (h d) -> p h d", d=64)
        if zero_mask:
            nc.vector.tensor_copy(out=vdst, in_=vsrc)
            nc.gpsimd.memset(ecol, 1.0)
        else:
            nc.vector.tensor_scalar_mul(out=vdst, in0=vsrc,
                                        scalar1=expm[:, kc:kc + 1])
            esrc = bass.AP(tensor=expm.tensor, offset=expm.offset + kc,
                           ap=[expm.ap[0], [0, 3]])
            nc.vector.tensor_copy(out=ecol, in_=esrc)
        nc.sync.dma_start(out=Vs[kc][0:64, :], in_=V[kc][64:128, :])
        nc.sync.dma_start(out=Vs[kc][64:128, :], in_=V[kc][0:64, :])

    def score_unit(h, J, g):
        """2 x N=512 score matmuls (kc = 2g, 2g+1) + one exp -> probs fp16."""
        if h < 2:
            KT, QT, prow = KT2, QT2, 64 * h
        else:
            KT, QT, prow = KTs, QTs, 0
        sc = ps.tile([128, 1024], F32, name=f"sc_{h}_{J}_{g}", tag="sc",
                     bufs=3)
        for jj in range(2):
            kc = 2 * g + jj
            nc.tensor.matmul(
                sc[:, 512 * jj:512 * (jj + 1)],
                KT[prow:prow + 64, 128 * kc:128 * (kc + 1)],
                QT[prow:prow + 64, 512 * J:512 * (J + 1)],
                start=True, stop=True)
        pt = probs_pool.tile([128, 1024], F16, name=f"pb_{h}_{J}_{g}",
                             tag="probs")
        nc.scalar.activation(pt, sc, EXP, scale=0.125)
        return pt

    def ctx_chain(h, J, probs, s):
        """One q-sub-tile's ctx accumulation + normalize + out store."""
        cx = ps.tile([128, 512], F32, name=f"cx_{h}_{J}_{s}", tag="mid", bufs=2)
        for g in range(NG):
            for jj in range(2):
                kc = 2 * g + jj
                nc.tensor.matmul(
                    cx[:, 0:65],
                    probs[g][:, 512 * jj + 128 * s:512 * jj + 128 * (s + 1)],
                    V[kc][:, 65 * h:65 * h + 65],
                    start=(kc == 0), stop=(kc == KC - 1))
        r = small.tile([128, 1], F32, name=f"r_{h}_{J}_{s}", tag="recip")
        nc.vector.reciprocal(r, cx[:, 64:65])
        u = 4 * J + s
        nc.vector.tensor_scalar_mul(
            out=out_tiles[u][:, 64 * h:64 * (h + 1)],
            in0=cx[:, 0:64], scalar1=r)
        out_written[u] += 1
        if out_written[u] == NHL:
            nc.sync.dma_start(out=out_d[128 * u:128 * (u + 1), :],
                              in_=out_tiles[u])

    # --- emission ------------------------------------------------------------
    # Unit kinds: ('01', J, kc) head-pair units (quadrant-paired matmuls) and
    # ('2', J, g) head-2 units.  Step-major order so steps complete early and
    # their ctx chains drain while later exps still run.  The softmax wall is
    # split ~2/3 ScalarE exp, ~1/3 DVE fast-exp.
    units = []
    for J in range(NQB):
        units += [('01', J, kc) for kc in range(KC)]
        units += [('2', J, g) for g in range(NG)]
    unit_idx = {u: i for i, u in enumerate(units)}
    emitted = set()
    probs01 = {J: [None] * KC for J in range(NQB)}
    probs2 = {J: [None] * NG for J in range(NQB)}
    done01 = {J: 0 for J in range(NQB)}
    done2 = {J: 0 for J in range(NQB)}
    pending = []          # ctx chains ready to emit
    vdone = [False]

    def _maybe_ready(kind, J):
        if not vdone[0]:
            return
        if kind == '01' and done01[J] == KC:
            for s in range(4):
                for var in range(2):
                    pending.append((ctx_pair01, (J, probs01[J], s, var)))
        if kind == '2' and done2[J] == NG:
            for s in range(4):
                pending.append((ctx_chain2, (J, probs2[J], s)))

    def emit_unit(kind, J, x):
        i = unit_idx[(kind, J, x)]
        use_dve = (i % 2 == 1) if i >= 64 else (i >= 8 and i % 5 in (1, 3))
        if kind == '01':
            probs01[J][x] = score_unit01(J, x, use_dve)
            done01[J] += 1
        else:
            probs2[J][x] = score_unit2(J, x, use_dve)
            done2[J] += 1
        emitted.add((kind, J, x))
        _maybe_ready(kind, J)

    def try_emit(q_m, k_m, solo_m, budget):
        for (kind, J, x) in units:
            if budget <= 0:
                return
            if (kind, J, x) in emitted:
                continue
            if kind == '01':
                ok = J <= q_m and x <= 4 * k_m + 3
            else:
                ok = J <= solo_m and 2 * x + 1 <= 4 * solo_m + 3
            if ok:
                emit_unit(kind, J, x)
                budget -= 1

    for m in range(4):
        x16s = x16s_first if m == 0 else load_x_block(m, cast_scalar)
        transpose_block(m, x16s)
        if m == 0:
            # keep the PE continuously busy while the last x16 tiles land;
            # an idle gap here resets the DVFS p-state to half clock.
            for i in range(12):
                warm = ps.tile([128, 128], F16, name=f"warm2_{i}", tag="mid",
                               bufs=2)
                nc.tensor.transpose(warm, ident16, ident16)
        proj_pair(1, KT2, m)   # K first: score units need K chunks earliest
        try_emit(m - 1, m, m - 1, 3)
        proj_pair(0, QT2, m)
        try_emit(m, m, m - 1, 3)
        if m == 0:
            emit_deferred_consts()
        proj_solo(m)
        try_emit(m, m, m, 3)
        for kc in range(4 * m, 4 * m + 4):
            build_v(kc)
        try_emit(m, m, m, 3)
    vdone[0] = True
    for J in range(NQB):  # steps fully emitted before V finished
        _maybe_ready('01', J)
        _maybe_ready('2', J)

    # Tail: remaining units interleaved with ctx chains of completed steps.
    # Pop harder when chains pile up so the post-last-exp drain stays short.
    for (kind, J, x) in units:
        if (kind, J, x) in emitted:
            continue
        if pending:
            fn, args = pending.pop(0)
            fn(*args)
        if len(pending) > 3:
            fn, args = pending.pop(0)
            fn(*args)
        emit_unit(kind, J, x)
    while pending:
        fn, args = pending.pop(0)
        fn(*args)

    for p in (ps, outp, small, probs_pool, persist, xpool, const):
        p.release()


_NC_CACHE = {}


def _get_nc(zero_mask: bool):
    if zero_mask not in _NC_CACHE:
        _NC_CACHE[zero_mask] = _build_kernel(zero_mask)
    return _NC_CACHE[zero_mask]


def kernel(hidden_states, attention_mask, Wq, bq, Wk, bk, Wv, bv, **run_kw):
    hidden_states = np.asarray(hidden_states, dtype=np.float32)
    attention_mask = np.asarray(attention_mask, dtype=np.float32)
    Wq, Wk, Wv = (np.asarray(a, dtype=np.float32) for a in (Wq, Wk, Wv))
    bq, bk, bv = (np.asarray(a, dtype=np.float32) for a in (bq, bk, bv))

    zero_mask = bool(np.all(attention_mask == 0.0))
    nc = _get_nc(zero_mask)
    in_maps = []
    for c in range(8):
        b, g = c // 4, c % 4
        cols = slice(192 * g, 192 * (g + 1))
        in_maps.append({
            "x": np.ascontiguousarray(hidden_states[b]),
            "wq": np.ascontiguousarray(Wq[:, cols]),
            "wk": np.ascontiguousarray(Wk[:, cols]),
            "wv": np.ascontiguousarray(Wv[:, cols]),
            "bq": np.ascontiguousarray(bq[cols]),
            "bk": np.ascontiguousarray(bk[cols]),
            "bv": np.ascontiguousarray(bv[cols]),
            "mask": np.ascontiguousarray(
                np.broadcast_to(attention_mask[b, 0, 0], (S,))),
        })
    res = run_bass_kernel_spmd(nc, in_maps, list(range(8)), **run_kw)
    out = np.empty((2, S, DM), dtype=np.float32)
    for c in range(8):
        b, g = c // 4, c % 4
        out[b, :, 192 * g:192 * (g + 1)] = res.results[c]["out"]
    if run_kw:
        return out, res
    return out
